# revision 19
# baseline (speedup 1.0000x reference)
"""Differential Multi-Query Attention — TRN2 Bass kernel, 8-core SPMD.

Sharding: tensor-parallel over the 16 query heads (2 heads per core).
MQA K/V (single head) is computed redundantly on every core. out_proj is
row-parallel: each core computes a partial [S, HID] output from its
256-wide slice of head dims; the all-reduce is the host-side gather sum.

Math notes (exact reformulations of the reference):
  * softmax without max-subtraction (scores ~ N(0,1), no overflow risk):
      a1 = exp(s1)/rowsum(exp(s1))
  * a = a1 - lam*a2 has rowsum exactly (1-lam), so the renorm divisor
    Z = (1-lam)+1e-8 is a constant -> folded into v_w on the host.
  * relu(a/Z) = relu(a)/Z since Z > 0.
  * 1/sqrt(head_dim) folded into q weights on the host (rope is a
    rotation, commutes with scaling).

Device layout: everything flows in "transposed" [feature, seq] form so
the tensor engine (which contracts over the partition dim) never needs
an activation transpose, except the post-softmax weights, which bounce
through DRAM once per (qb, head) as a single batched xbar-transpose DMA
(the xbar transpose is only correct with a DRAM source on HW).

v2 changes vs the first working version:
  * batched transposes: one store + one transpose DMA per (qb, head)
    instead of per 128x128 block (288 DMAs -> ~66), spread across the
    scalar (store) and sync (transpose) HWDGE queues.
  * stage A reordered t-outer so consecutive matmuls share lhsT
    (16 LDWEIGHTS per projection target instead of 64).
  * v projected first so its DRAM-bounce transpose hides under the
    remaining projections; k second so qb0 scores can start early.
  * relu done in-place on the DVE (frees the ACT engine), attnT copies
    on the Pool engine, out_proj partials stored as bf16 (halves the
    output DMA traffic; host accumulates in f32).
  * 2-deep qb software pipeline: scores(qb) emitted before AV/out_proj
    of qb-2 so the PE always has independent work while the weight
    strips round-trip through DRAM.
"""

import math
from contextlib import ExitStack

import numpy as np

import concourse.bass as bass
import concourse.bacc as bacc
import concourse.tile as tile
from concourse import mybir
from concourse.bass_utils import run_bass_kernel_spmd

S = 2048          # sequence length
HID = 2048        # hidden dim
HEADS = 16
D = 128           # head dim
NCORES = 8
HPC = HEADS // NCORES   # heads per core = 2
LAM = 0.5
NQB = S // 128    # query blocks of 128
NHT = HID // 128  # hidden-dim tiles of 128

F32 = mybir.dt.float32
BF16 = mybir.dt.bfloat16
AF = mybir.ActivationFunctionType
OP = mybir.AluOpType

NEG_MASK = -1.0e9


def _emit(ctx, tc, xT, wq1, wq2, wk, wv, cosT, sinT, prot, mneg, ow, y, klens):
    nc = tc.nc

    # ---------------- persistent tiles ----------------
    persist = ctx.enter_context(tc.tile_pool(name="persist", bufs=1))
    cos_s = persist.tile([128, S], BF16, tag="cos")
    sin_s = persist.tile([128, S], BF16, tag="sin")

    # roped projections, [d=128, S] each (transposed form)
    q1r = [persist.tile([128, S], BF16, name=f"q1r{h}", tag=f"q1r{h}") for h in range(HPC)]
    q2r = [persist.tile([128, S], BF16, name=f"q2r{h}", tag=f"q2r{h}") for h in range(HPC)]
    kr = persist.tile([128, S], BF16, tag="kr")
    vt_bf = persist.tile([128, S], BF16, tag="vt_bf")         # v^T, bf16
    v_nat = persist.tile([128, NQB, 128], BF16, tag="v_nat")  # v natural [k, d]

    ow_s = persist.tile([128, HPC, HID], BF16, tag="ow")
    mneg_s = persist.tile([128, S], F32, tag="mneg")

    dram = ctx.enter_context(tc.tile_pool(name="dram", bufs=1, space="DRAM"))

    # ---------------- stage A: projections + rope ----------------
    with tc.tile_pool(name="wpool", bufs=1) as wp, \
         tc.tile_pool(name="xpool", bufs=1) as xp, \
         tc.tile_pool(name="ropetmp", bufs=2) as rtp, \
         tc.tile_pool(name="projpsum", bufs=1, space="PSUM") as pp, \
         tc.tile_pool(name="rotpsum", bufs=1, space="PSUM") as rpp, \
         tc.tile_pool(name="warmpsum", bufs=1, space="PSUM") as wmp:
        wq1_s = wp.tile([128, NHT, HPC * D], BF16, tag="wq1")
        wq2_s = wp.tile([128, NHT, HPC * D], BF16, tag="wq2")
        wk_s = wp.tile([128, NHT, D], BF16, tag="wk")
        wv_s = wp.tile([128, NHT, D], BF16, tag="wv")
        prot_s = wp.tile([128, D], BF16, tag="prot")
        # load order: prot+cos first (feed the PE warm-up matmuls), then
        # the projection weights, then x; ow/mneg (stage B only) last
        nc.sync.dma_start(out=prot_s, in_=prot)
        nc.sync.dma_start(out=cos_s, in_=cosT)
        nc.scalar.dma_start(out=wv_s, in_=wv.rearrange("(t p) d -> p t d", p=128))
        nc.scalar.dma_start(out=wk_s, in_=wk.rearrange("(t p) d -> p t d", p=128))
        nc.scalar.dma_start(out=wq1_s, in_=wq1.rearrange("(t p) d -> p t d", p=128))
        nc.scalar.dma_start(out=wq2_s, in_=wq2.rearrange("(t p) d -> p t d", p=128))
        nc.sync.dma_start(out=sin_s, in_=sinT)

        # x, transposed [hid, s]: first t-slices individually (the t=0
        # accumulation step gates everything), then larger groups
        xt = xp.tile([128, NHT, S], BF16, tag="xt")
        xin = xT.rearrange("(t p) s -> p t s", p=128)
        xgroups = [(0, 1), (1, 2), (2, 4), (4, 8), (8, 12), (12, 16)]
        for gi, (t0, t1) in enumerate(xgroups):
            eng = nc.sync if gi % 2 == 0 else nc.scalar
            eng.dma_start(out=xt[:, t0:t1, :], in_=xin[:, t0:t1, :])
        nc.scalar.dma_start(out=ow_s, in_=ow.rearrange("(h p) e -> p h e", p=128))
        nc.scalar.dma_start(out=mneg_s, in_=mneg)

        # PE warm-up: junk matmuls on already-loaded tiles keep the HAM
        # activity window busy while x streams in, so the real projection
        # matmuls start at 2.4 GHz instead of 1.2 (throwaway psum).
        warm_ps = wmp.tile([128, 512], F32, tag="warm")
        for i in range(32):
            nc.tensor.matmul(warm_ps, lhsT=prot_s,
                             rhs=cos_s[:, (i % 3) * 512:(i % 3) * 512 + 512],
                             start=True, stop=True, skip_group_check=True)

        # targets in emission order: v first (its DRAM-bounce transpose
        # hides under the rest), k second (qb0 scores need it first)
        targets = [(wv_s, 0, None, False), (wk_s, 0, kr, True)]
        for h in range(HPC):
            targets.append((wq1_s, h * D, q1r[h], True))
            targets.append((wq2_s, h * D, q2r[h], True))

        def emit_post(post):
            (ps_pair, dest, do_rope) = post
            for c in range(2):
                sl = slice(c * 1024, (c + 1) * 1024)
                ps = ps_pair[c]
                if do_rope:
                    # rope in [d, s] layout: out = q*cos + rot_half(q)*sin.
                    # rot_half is a signed half-swap along the PARTITION dim;
                    # cross-partition reads are illegal on the vector engines,
                    # so apply it as a signed permutation matmul on PE.
                    q_sb = rtp.tile([128, 1024], BF16, tag="qsb")
                    nc.scalar.copy(out=q_sb, in_=ps)
                    rot_ps = rpp.tile([128, 1024], F32, tag="rot")
                    for sub in range(2):
                        ssl = slice(sub * 512, (sub + 1) * 512)
                        nc.tensor.matmul(rot_ps[:, ssl], lhsT=prot_s,
                                         rhs=q_sb[:, ssl], start=True, stop=True)
                    m = rtp.tile([128, 1024], BF16, tag="m")
                    nc.vector.tensor_tensor(out=m, in0=q_sb, in1=cos_s[:, sl], op=OP.mult)
                    n = rtp.tile([128, 1024], BF16, tag="n")
                    nc.vector.tensor_tensor(out=n, in0=rot_ps, in1=sin_s[:, sl], op=OP.mult)
                    nc.vector.tensor_tensor(out=dest[:, sl], in0=m, in1=n, op=OP.add)
                else:
                    nc.scalar.copy(out=vt_bf[:, sl], in_=ps)  # cast f32 -> bf16

        pending_post = None
        vt_dram = dram.tile([128, S], BF16, tag="vt_dram")
        for ti, (w_s, d0, dest, do_rope) in enumerate(targets):
            ps_pair = [pp.tile([128, 1024], F32, name=f"ps{c}", tag=f"ps{c}")
                       for c in range(2)]
            for t in range(NHT):
                for c in range(2):
                    for sub in range(2):
                        sl = slice(c * 1024 + sub * 512, c * 1024 + (sub + 1) * 512)
                        psl = slice(sub * 512, (sub + 1) * 512)
                        nc.tensor.matmul(
                            ps_pair[c][:, psl],
                            lhsT=w_s[:, t, d0:d0 + D],
                            rhs=xt[:, t, sl],
                            start=(t == 0),
                            stop=(t == NHT - 1),
                        )
            if pending_post is not None:
                emit_post(pending_post)
            if ti == 1:
                # v post-ops were just emitted (ti-1 == 0 == v): bounce v^T
                # through DRAM as one batched xbar transpose
                nc.scalar.dma_start(out=vt_dram, in_=vt_bf)
                nc.sync.dma_start(out=v_nat, in_=vt_dram, transpose=True)
            pending_post = (ps_pair, dest, do_rope)
        emit_post(pending_post)

    # ---------------- stage B: attention + out_proj ----------------
    with tc.tile_pool(name="strips", bufs=2) as sp, \
         tc.tile_pool(name="smallp", bufs=4) as smp, \
         tc.tile_pool(name="atp", bufs=3) as atp, \
         tc.tile_pool(name="yp", bufs=2) as yp, \
         tc.tile_pool(name="wdram", bufs=3, space="DRAM") as wdp, \
         tc.tile_pool(name="spsum", bufs=2, space="PSUM") as spp, \
         tc.tile_pool(name="ypsum", bufs=2, space="PSUM") as ypp, \
         tc.tile_pool(name="apsum", bufs=2, space="PSUM") as app:

        def emit_scores(qb):
            """scores -> exp -> renorm -> relu -> DRAM bounce -> aT tile.
            Both heads' weight strips land contiguously in one DRAM tile so
            a single xbar transpose yields [k, 2*nkt blocks, q]; AV then
            runs 256-wide with a strided rhs covering both heads."""
            klen = klens[qb]
            nkt = klen // 128
            nch = (klen + 1023) // 1024
            w_dram = wdp.tile([128, 2 * S], BF16, tag="w_dram")
            for h in range(HPC):
                p1 = sp.tile([128, S], BF16, tag=f"p1h{h}")
                p2 = sp.tile([128, S], BF16, tag=f"p2h{h}")
                r1c = smp.tile([128, 2], F32, tag=f"r1c{h}")
                r2c = smp.tile([128, 2], F32, tag=f"r2c{h}")
                q1T = q1r[h][:, qb * 128:(qb + 1) * 128]
                q2T = q2r[h][:, qb * 128:(qb + 1) * 128]
                for (qT, pstrip, rc) in ((q1T, p1, r1c), (q2T, p2, r2c)):
                    for c in range(nch):
                        k0 = c * 1024
                        kc = min(1024, klen - k0)
                        sps = spp.tile([128, 1024], F32, tag="s")
                        for sub in range(0, kc, 512):
                            sw = min(512, kc - sub)
                            nc.tensor.matmul(
                                sps[:, sub:sub + sw],
                                lhsT=qT,
                                rhs=kr[:, k0 + sub:k0 + sub + sw],
                                start=True,
                                stop=True,
                            )
                        if c == nch - 1:
                            # mask for the last (possibly partial) k block
                            dc = kc - 128
                            nc.vector.tensor_tensor(
                                out=sps[:, dc:dc + 128],
                                in0=sps[:, dc:dc + 128],
                                in1=mneg_s[:, qb * 128:(qb + 1) * 128],
                                op=OP.add,
                            )
                        nc.scalar.activation(
                            out=pstrip[:, k0:k0 + kc],
                            in_=sps[:, :kc],
                            func=AF.Exp,
                            accum_out=rc[:, c:c + 1],
                        )
                # rowsums. w = relu(p1/r1 - lam*p2/r2) is refactored as
                #   w = c1 * relu(p1 + c2r*p2),  c1 = 1/r1, c2r = -lam*r1/r2
                # so the whole renorm is ONE full-width stt pass plus a
                # fused relu+scale (ACT for h0, DVE two-scalar op for h1).
                c1 = smp.tile([128, 1], F32, tag=f"c1h{h}")
                c2r = smp.tile([128, 1], F32, tag=f"c2rh{h}")
                rec2 = smp.tile([128, 1], F32, tag=f"rec2h{h}")
                if nch > 1:
                    nc.vector.tensor_tensor(
                        out=r1c[:, 0:1], in0=r1c[:, 0:1], in1=r1c[:, 1:2], op=OP.add
                    )
                    nc.vector.tensor_tensor(
                        out=r2c[:, 0:1], in0=r2c[:, 0:1], in1=r2c[:, 1:2], op=OP.add
                    )
                nc.vector.reciprocal(out=c1, in_=r1c[:, 0:1])
                nc.vector.reciprocal(out=rec2, in_=r2c[:, 0:1])
                nc.vector.scalar_tensor_tensor(
                    out=c2r, in0=rec2, scalar=-LAM, in1=r1c[:, 0:1],
                    op0=OP.mult, op1=OP.mult,
                )
                nc.vector.scalar_tensor_tensor(
                    out=p2[:, :klen],
                    in0=p2[:, :klen],
                    scalar=c2r,
                    in1=p1[:, :klen],
                    op0=OP.mult,
                    op1=OP.add,
                )
                if h == 0:
                    nc.scalar.activation(out=p2[:, :klen], in_=p2[:, :klen],
                                         func=AF.Relu, scale=c1)
                else:
                    nc.vector.tensor_scalar(
                        out=p2[:, :klen], in0=p2[:, :klen],
                        scalar1=0.0, scalar2=c1, op0=OP.max, op1=OP.mult,
                    )
                # store this head's strip into its half of the shared DRAM
                # tile (gpsimd SWDGE queue — keeps scalar/sync free)
                nc.gpsimd.dma_start(out=w_dram[:, h * klen:h * klen + klen],
                                    in_=p2[:, :klen])
            # one batched xbar transpose for both heads: blocks 0..nkt-1 are
            # h0, nkt..2nkt-1 are h1
            aT = atp.tile([128, 2 * NQB, 128], BF16, tag="aT")
            nc.sync.dma_start(out=aT[:, :2 * nkt, :], in_=w_dram[:, :2 * klen],
                              transpose=True)
            return aT

        def emit_av_outproj(qb, aT):
            klen = klens[qb]
            nkt = klen // 128
            # attn_out^T[d, q] = sum_k v[k, d] * w^T[k, q], both heads in one
            # 256-wide moving operand: rhs = blocks (kt, nkt+kt) via stride
            avps = app.tile([128, 2, 128], F32, tag="av")
            aTb = aT[:, :2 * nkt, :].rearrange("p (h t) q -> p h t q", h=2)
            for kt in range(nkt):
                nc.tensor.matmul(
                    avps,
                    lhsT=v_nat[:, kt, :],
                    rhs=aTb[:, :, kt, :],
                    start=(kt == 0),
                    stop=(kt == nkt - 1),
                )
            at_s = smp.tile([128, 2, 128], BF16, name="attnT", tag="attnT")
            nc.vector.tensor_copy(out=at_s, in_=avps)
            # row-parallel out_proj partial: y[qb] = sum_h attnT_h.T @ ow_h
            ysb = yp.tile([128, HID], BF16, tag="ysb")
            for ec in range(4):
                yps = ypp.tile([128, 512], F32, tag="yps")
                e0 = ec * 512
                for h in range(HPC):
                    nc.tensor.matmul(
                        yps,
                        lhsT=at_s[:, h, :],
                        rhs=ow_s[:, h, e0:e0 + 512],
                        start=(h == 0),
                        stop=(h == HPC - 1),
                    )
                if ec % 2 == 0:
                    nc.scalar.copy(out=ysb[:, e0:e0 + 512], in_=yps)
                else:
                    nc.vector.tensor_copy(out=ysb[:, e0:e0 + 512], in_=yps)
            nc.sync.dma_start(out=y[qb * 128:(qb + 1) * 128, :], in_=ysb)

        DEFER = 2
        pending = []
        for qb in range(NQB):
            aTs = emit_scores(qb)
            pending.append((qb, aTs))
            if len(pending) > DEFER:
                emit_av_outproj(*pending.pop(0))
        for item in pending:
            emit_av_outproj(*item)


def _build(klens):
    nc = bacc.Bacc("TRN2", target_bir_lowering=False, debug=False)
    xT = nc.dram_tensor("xT", [HID, S], BF16, kind="ExternalInput").ap()
    wq1 = nc.dram_tensor("wq1", [HID, HPC * D], BF16, kind="ExternalInput").ap()
    wq2 = nc.dram_tensor("wq2", [HID, HPC * D], BF16, kind="ExternalInput").ap()
    wk = nc.dram_tensor("wk", [HID, D], BF16, kind="ExternalInput").ap()
    wv = nc.dram_tensor("wv", [HID, D], BF16, kind="ExternalInput").ap()
    cosT = nc.dram_tensor("cosT", [D, S], BF16, kind="ExternalInput").ap()
    prot = nc.dram_tensor("prot", [D, D], BF16, kind="ExternalInput").ap()
    sinT = nc.dram_tensor("sinT", [D, S], BF16, kind="ExternalInput").ap()
    mneg = nc.dram_tensor("mneg", [128, S], F32, kind="ExternalInput").ap()
    ow = nc.dram_tensor("ow", [HPC * D, HID], BF16, kind="ExternalInput").ap()
    y = nc.dram_tensor("y", [S, HID], BF16, kind="ExternalOutput").ap()
    with ExitStack() as ctx:
        tc = ctx.enter_context(tile.TileContext(nc))
        _emit(ctx, tc, xT, wq1, wq2, wk, wv, cosT, sinT, prot, mneg, ow, y, klens)
    nc.compile()
    return nc


_RUNNER_CACHE = {}
LAST_RUN = None
LAST_EXEC = None  # (runner, dev_args) for timing reuse


class _Runner:
    """Mirrors bass2jax.run_bass_via_pjrt's multi-core path, but caches the
    jitted executable and keeps inputs reusable (no donation) so repeated
    timed executions don't recompile or re-upload."""

    def __init__(self, nc, n_cores):
        import jax
        from jax.sharding import Mesh, PartitionSpec
        from jax.experimental.shard_map import shard_map
        from concourse import bass2jax, mybir as mb

        bass2jax.install_neuronx_cc_hook()
        self.nc = nc
        self.n_cores = n_cores
        partition_name = (
            nc.partition_id_tensor.name if nc.partition_id_tensor else None
        )
        in_names, out_names, out_avals, zero_outs = [], [], [], []
        for alloc in nc.m.functions[0].allocations:
            if not isinstance(alloc, mb.MemoryLocationSet):
                continue
            name = alloc.memorylocations[0].name
            if alloc.kind == "ExternalInput":
                if name != partition_name:
                    in_names.append(name)
            elif alloc.kind == "ExternalOutput":
                out_names.append(name)
                shape = tuple(alloc.tensor_shape)
                dtype = mb.dt.np(alloc.dtype)
                out_avals.append(jax.core.ShapedArray(shape, dtype))
                zero_outs.append(np.zeros(shape, dtype))
        self.in_names = list(in_names)
        self.out_names = out_names
        self.out_avals = out_avals
        self.zero_outs = zero_outs
        n_params = len(in_names)
        all_names = list(in_names + out_names)
        if partition_name is not None:
            all_names.append(partition_name)
        all_names = tuple(all_names)

        def _body(*args):
            operands = list(args)
            if partition_name is not None:
                operands.append(bass2jax.partition_id_tensor())
            outs = bass2jax._bass_exec_p.bind(
                *operands,
                out_avals=tuple(out_avals),
                in_names=all_names,
                out_names=tuple(out_names),
                lowering_input_output_aliases=(),
                sim_require_finite=True,
                sim_require_nnan=True,
                nc=nc,
            )
            return tuple(outs)

        self._body = _body
        devices = jax.devices()[:n_cores]
        self.mesh = Mesh(np.asarray(devices), ("core",))
        self.pspec = PartitionSpec("core")
        in_specs = (self.pspec,) * (n_params + len(out_names))
        out_specs = (self.pspec,) * len(out_names)
        self.fn = jax.jit(
            shard_map(_body, mesh=self.mesh, in_specs=in_specs,
                      out_specs=out_specs, check_rep=False),
            keep_unused=True,
        )

    def device_args(self, in_maps):
        import jax
        from jax.sharding import NamedSharding

        sharding = NamedSharding(self.mesh, self.pspec)
        concat = [
            np.concatenate([np.asarray(m[name]) for m in in_maps], axis=0)
            for name in self.in_names
        ]
        concat += [
            np.zeros((self.n_cores * z.shape[0], *z.shape[1:]), z.dtype)
            for z in self.zero_outs
        ]
        return [jax.device_put(a, sharding) for a in concat]

    def run(self, dev_args):
        import jax

        outs = self.fn(*dev_args)
        jax.block_until_ready(outs)
        return [
            {
                name: np.asarray(outs[i]).reshape(
                    self.n_cores, *self.out_avals[i].shape)[c]
                for i, name in enumerate(self.out_names)
            }
            for c in range(self.n_cores)
        ]


def _get_runner(klens):
    key = tuple(klens)
    if key not in _RUNNER_CACHE:
        _RUNNER_CACHE[key] = _Runner(_build(klens), NCORES)
    return _RUNNER_CACHE[key]


def measure_hw(outdir="/tmp/kprof"):
    """Honest per-execution HW time: capture a neuron-profile NTFF of one
    kernel execution via the axon profiling ABI and report the profiler's
    exec time for the slowest profiled core."""
    import ctypes
    import glob
    import os
    import jax

    runner, dev_args = LAST_EXEC
    os.makedirs(outdir, exist_ok=True)
    for f in glob.glob(os.path.join(outdir, "*")):
        os.remove(f)

    lib = ctypes.CDLL('/opt/axon/libaxon_pjrt.so')
    lib.axon_start_nrt_profile.argtypes = [
        ctypes.POINTER(ctypes.c_int64), ctypes.c_size_t]
    lib.axon_start_nrt_profile.restype = ctypes.c_int64
    lib.axon_stop_nrt_profile.argtypes = [ctypes.c_char_p]
    lib.axon_stop_nrt_profile.restype = ctypes.c_int64

    # warm once (clocks/рstate), then profile a single execution
    jax.block_until_ready(runner.fn(*dev_args))
    rc = lib.axon_start_nrt_profile(None, 0)
    assert rc == 0, f"axon_start_nrt_profile rc={rc}"
    outs = runner.fn(*dev_args)
    jax.block_until_ready(outs)
    n = lib.axon_stop_nrt_profile(outdir.encode())
    assert n > 0, f"profile produced no files (rc={n})"

    from gauge.profiler import Profile
    from concourse._compat import FishPath

    prof = Profile(
        profile_path=FishPath(outdir),
        kernel_dev_mode=True,
        profile_on_exit=False,
        bass_kernel=runner.nc.m,
        offline_processing=True,
        fname="*",
    )
    results = prof.to_perfetto(model_index=(0,))
    exec_ns = max(r.exec_time_ns for r in results)
    trace_path = results[0].trace_path
    return exec_ns, trace_path


def _prep_mask(mask):
    """Per query-block: attended k extent (klen) and the additive mask for
    the last 128-wide k block. Requires every non-final block in range to
    be all-True (holds for causal and for all-ones masks)."""
    mask = np.asarray(mask).astype(bool)
    klens = []
    mneg = np.zeros((128, S), np.float32)
    for qb in range(NQB):
        rows = mask[qb * 128:(qb + 1) * 128, :]
        any_col = rows.any(axis=0)
        assert any_col.any(), f"query block {qb} attends nothing"
        last = int(np.nonzero(any_col)[0][-1])
        nkt = last // 128 + 1
        klen = nkt * 128
        klens.append(klen)
        blk = rows[:, (nkt - 1) * 128:klen]
        mneg[:, qb * 128:(qb + 1) * 128] = np.where(blk, 0.0, NEG_MASK)
        inner = rows[:, :(nkt - 1) * 128]
        if not inner.all():
            raise NotImplementedError(
                "mask has partial blocks before the final attended block; "
                "only causal / all-ones style masks are supported"
            )
    return klens, mneg


def host_prep(x, freqs_cos, freqs_sin, mask, q1_w, q2_w, k_w, v_w, out_w):
    """Host-side input marshalling: transpose/fold/shard. Returns
    (klens, in_maps)."""
    x = np.asarray(x, np.float32)
    assert x.shape == (1, S, HID)
    xT = np.ascontiguousarray(x[0].T)
    scale = 1.0 / math.sqrt(D)
    Z = (1.0 - LAM) + 1e-8

    cosT = np.ascontiguousarray(np.asarray(freqs_cos, np.float32).T)
    sinT = np.ascontiguousarray(np.asarray(freqs_sin, np.float32).T)
    # signed rotate-half as a matmul: rot = protM @ q with
    # protM[d, d+64] = -1 (d<64), protM[d, d-64] = +1 (d>=64); lhsT = protM.T
    protM = np.zeros((D, D), np.float32)
    for d in range(64):
        protM[d, d + 64] = -1.0
        protM[d + 64, d] = 1.0
    protT = np.ascontiguousarray(protM.T)

    klens, mneg = _prep_mask(mask)

    q1_w = np.asarray(q1_w, np.float32) * scale
    q2_w = np.asarray(q2_w, np.float32) * scale
    k_w = np.ascontiguousarray(np.asarray(k_w, np.float32))
    v_w = np.ascontiguousarray(np.asarray(v_w, np.float32) / Z)
    out_w = np.asarray(out_w, np.float32)

    import ml_dtypes
    bf = ml_dtypes.bfloat16
    xT = xT.astype(bf)
    k_w = k_w.astype(bf)
    v_w = v_w.astype(bf)
    protT = protT.astype(bf)
    cosT = cosT.astype(bf)
    sinT = sinT.astype(bf)
    in_maps = []
    for c in range(NCORES):
        h0 = c * HPC * D
        in_maps.append({
            "xT": xT,
            "wq1": np.ascontiguousarray(q1_w[:, h0:h0 + HPC * D]).astype(bf),
            "wq2": np.ascontiguousarray(q2_w[:, h0:h0 + HPC * D]).astype(bf),
            "wk": k_w,
            "wv": v_w,
            "cosT": cosT,
            "sinT": sinT,
            "prot": protT,
            "mneg": mneg,
            "ow": np.ascontiguousarray(out_w[h0:h0 + HPC * D, :]).astype(bf),
        })
    return klens, in_maps


def kernel(x, freqs_cos, freqs_sin, mask, q1_w, q2_w, k_w, v_w, out_w):
    global LAST_RUN, LAST_EXEC
    klens, in_maps = host_prep(
        x, freqs_cos, freqs_sin, mask, q1_w, q2_w, k_w, v_w, out_w)
    runner = _get_runner(klens)
    dev_args = runner.device_args(in_maps)
    LAST_EXEC = (runner, dev_args)
    results = runner.run(dev_args)
    LAST_RUN = results
    y = results[0]["y"].astype(np.float32)
    for c in range(1, NCORES):
        y = y + results[c]["y"].astype(np.float32)
    return y.reshape(1, S, HID)


# revision 25
# speedup vs baseline: 1.0216x; 1.0216x over previous
"""Differential Multi-Query Attention — TRN2 Bass kernel, 8-core SPMD.

Sharding: tensor-parallel over the 16 query heads (2 heads per core).
MQA K/V (single head) is computed redundantly on every core. out_proj is
row-parallel: each core computes a partial [S, HID] output from its
256-wide slice of head dims; the all-reduce is the host-side gather sum.

Math notes (exact reformulations of the reference):
  * softmax without max-subtraction (scores ~ N(0,1), no overflow risk):
      a1 = exp(s1)/rowsum(exp(s1))
  * a = a1 - lam*a2 has rowsum exactly (1-lam), so the renorm divisor
    Z = (1-lam)+1e-8 is a constant -> folded into v_w on the host.
  * relu(a/Z) = relu(a)/Z since Z > 0.
  * 1/sqrt(head_dim) folded into q weights on the host (rope is a
    rotation, commutes with scaling).

Device layout: everything flows in "transposed" [feature, seq] form so
the tensor engine (which contracts over the partition dim) never needs
an activation transpose, except the post-softmax weights, which bounce
through DRAM once per (qb, head) as a single batched xbar-transpose DMA
(the xbar transpose is only correct with a DRAM source on HW).

v2 changes vs the first working version:
  * batched transposes: one store + one transpose DMA per (qb, head)
    instead of per 128x128 block (288 DMAs -> ~66), spread across the
    scalar (store) and sync (transpose) HWDGE queues.
  * stage A reordered t-outer so consecutive matmuls share lhsT
    (16 LDWEIGHTS per projection target instead of 64).
  * v projected first so its DRAM-bounce transpose hides under the
    remaining projections; k second so qb0 scores can start early.
  * relu done in-place on the DVE (frees the ACT engine), attnT copies
    on the Pool engine, out_proj partials stored as bf16 (halves the
    output DMA traffic; host accumulates in f32).
  * 2-deep qb software pipeline: scores(qb) emitted before AV/out_proj
    of qb-2 so the PE always has independent work while the weight
    strips round-trip through DRAM.
"""

import math
from contextlib import ExitStack

import numpy as np

import concourse.bass as bass
import concourse.bacc as bacc
import concourse.tile as tile
from concourse import mybir
from concourse.bass_utils import run_bass_kernel_spmd

S = 2048          # sequence length
HID = 2048        # hidden dim
HEADS = 16
D = 128           # head dim
NCORES = 8
HPC = HEADS // NCORES   # heads per core = 2
LAM = 0.5
NQB = S // 128    # query blocks of 128
NHT = HID // 128  # hidden-dim tiles of 128

F32 = mybir.dt.float32
BF16 = mybir.dt.bfloat16
AF = mybir.ActivationFunctionType
OP = mybir.AluOpType

NEG_MASK = -1.0e9


def _emit(ctx, tc, xT, wq1, wq2, wk, wv, cosT, sinT, prot, mneg, ow, y, klens):
    nc = tc.nc

    # ---------------- persistent tiles ----------------
    persist = ctx.enter_context(tc.tile_pool(name="persist", bufs=1))
    cos_s = persist.tile([128, S], BF16, tag="cos")
    sin_s = persist.tile([128, S], BF16, tag="sin")

    # roped projections, [d=128, S] each (transposed form)
    q1r = [persist.tile([128, S], BF16, name=f"q1r{h}", tag=f"q1r{h}") for h in range(HPC)]
    q2r = [persist.tile([128, S], BF16, name=f"q2r{h}", tag=f"q2r{h}") for h in range(HPC)]
    kr = persist.tile([128, S], BF16, tag="kr")
    vt_bf = persist.tile([128, S], BF16, tag="vt_bf")         # v^T, bf16
    v_nat = persist.tile([128, NQB, 128], BF16, tag="v_nat")  # v natural [k, d]

    ow_s = persist.tile([128, HPC, HID], BF16, tag="ow")
    mneg_s = persist.tile([128, S], F32, tag="mneg")

    dram = ctx.enter_context(tc.tile_pool(name="dram", bufs=1, space="DRAM"))

    # ---------------- stage A: projections + rope ----------------
    with tc.tile_pool(name="wpool", bufs=1) as wp, \
         tc.tile_pool(name="xpool", bufs=1) as xp, \
         tc.tile_pool(name="ropetmp", bufs=2) as rtp, \
         tc.tile_pool(name="projpsum", bufs=1, space="PSUM") as pp, \
         tc.tile_pool(name="rotpsum", bufs=1, space="PSUM") as rpp, \
         tc.tile_pool(name="warmpsum", bufs=1, space="PSUM") as wmp:
        wq1_s = wp.tile([128, NHT, HPC * D], BF16, tag="wq1")
        wq2_s = wp.tile([128, NHT, HPC * D], BF16, tag="wq2")
        wk_s = wp.tile([128, NHT, D], BF16, tag="wk")
        wv_s = wp.tile([128, NHT, D], BF16, tag="wv")
        prot_s = wp.tile([128, D], BF16, tag="prot")
        # load order: prot+cos first (feed the PE warm-up matmuls), then
        # the projection weights, then x; ow/mneg (stage B only) last
        nc.sync.dma_start(out=prot_s, in_=prot)
        nc.sync.dma_start(out=cos_s, in_=cosT)
        nc.scalar.dma_start(out=wv_s, in_=wv.rearrange("(t p) d -> p t d", p=128))
        nc.scalar.dma_start(out=wk_s, in_=wk.rearrange("(t p) d -> p t d", p=128))
        nc.scalar.dma_start(out=wq1_s, in_=wq1.rearrange("(t p) d -> p t d", p=128))
        nc.scalar.dma_start(out=wq2_s, in_=wq2.rearrange("(t p) d -> p t d", p=128))
        nc.scalar.dma_start(out=sin_s, in_=sinT)

        # x, transposed [hid, s]: all on the sync queue in t order (the
        # projection accumulation consumes slices in t order), weights on
        # scalar -- the two queues never block each other
        xt = xp.tile([128, NHT, S], BF16, tag="xt")
        xin = xT.rearrange("(t p) s -> p t s", p=128)
        xgroups = [(0, 1), (1, 2), (2, 4), (4, 8), (8, 12), (12, 16)]
        for (t0, t1) in xgroups:
            nc.sync.dma_start(out=xt[:, t0:t1, :], in_=xin[:, t0:t1, :])
        nc.scalar.dma_start(out=ow_s, in_=ow.rearrange("(h p) e -> p h e", p=128))
        nc.scalar.dma_start(out=mneg_s, in_=mneg)

        # PE warm-up: junk matmuls on already-loaded tiles keep the HAM
        # activity window busy while x streams in, so the real projection
        # matmuls start at 2.4 GHz instead of 1.2 (throwaway psum).
        warm_ps = wmp.tile([128, 512], F32, tag="warm")
        for i in range(48):
            nc.tensor.matmul(warm_ps, lhsT=prot_s,
                             rhs=cos_s[:, (i % 3) * 512:(i % 3) * 512 + 512],
                             start=True, stop=True, skip_group_check=True)

        # targets in emission order: v first (its DRAM-bounce transpose
        # hides under the rest), k second (qb0 scores need it first)
        targets = [(wv_s, 0, None, False), (wk_s, 0, kr, True)]
        for h in range(HPC):
            targets.append((wq1_s, h * D, q1r[h], True))
            targets.append((wq2_s, h * D, q2r[h], True))

        def emit_post(post):
            (ps_pair, dest, do_rope) = post
            for c in range(2):
                sl = slice(c * 1024, (c + 1) * 1024)
                ps = ps_pair[c]
                if do_rope:
                    # rope in [d, s] layout: out = q*cos + rot_half(q)*sin.
                    # rot_half is a signed half-swap along the PARTITION dim;
                    # cross-partition reads are illegal on the vector engines,
                    # so apply it as a signed permutation matmul on PE.
                    q_sb = rtp.tile([128, 1024], BF16, tag="qsb")
                    nc.scalar.copy(out=q_sb, in_=ps)
                    rot_ps = rpp.tile([128, 1024], F32, tag="rot")
                    for sub in range(2):
                        ssl = slice(sub * 512, (sub + 1) * 512)
                        nc.tensor.matmul(rot_ps[:, ssl], lhsT=prot_s,
                                         rhs=q_sb[:, ssl], start=True, stop=True)
                    m = rtp.tile([128, 1024], BF16, tag="m")
                    nc.vector.tensor_tensor(out=m, in0=q_sb, in1=cos_s[:, sl], op=OP.mult)
                    n = rtp.tile([128, 1024], BF16, tag="n")
                    nc.vector.tensor_tensor(out=n, in0=rot_ps, in1=sin_s[:, sl], op=OP.mult)
                    nc.vector.tensor_tensor(out=dest[:, sl], in0=m, in1=n, op=OP.add)
                else:
                    nc.scalar.copy(out=vt_bf[:, sl], in_=ps)  # cast f32 -> bf16

        pending_post = None
        vt_dram = dram.tile([128, S], BF16, tag="vt_dram")
        for ti, (w_s, d0, dest, do_rope) in enumerate(targets):
            ps_pair = [pp.tile([128, 1024], F32, name=f"ps{c}", tag=f"ps{c}")
                       for c in range(2)]
            for t in range(NHT):
                for c in range(2):
                    for sub in range(2):
                        sl = slice(c * 1024 + sub * 512, c * 1024 + (sub + 1) * 512)
                        psl = slice(sub * 512, (sub + 1) * 512)
                        nc.tensor.matmul(
                            ps_pair[c][:, psl],
                            lhsT=w_s[:, t, d0:d0 + D],
                            rhs=xt[:, t, sl],
                            start=(t == 0),
                            stop=(t == NHT - 1),
                        )
            if pending_post is not None:
                emit_post(pending_post)
            if ti == 1:
                # v post-ops were just emitted (ti-1 == 0 == v): bounce v^T
                # through DRAM as one batched xbar transpose
                nc.scalar.dma_start(out=vt_dram, in_=vt_bf)
                nc.sync.dma_start(out=v_nat, in_=vt_dram, transpose=True)
            pending_post = (ps_pair, dest, do_rope)
        emit_post(pending_post)

    # ---------------- stage B: attention + out_proj ----------------
    with tc.tile_pool(name="strips", bufs=2) as sp, \
         tc.tile_pool(name="smallp", bufs=4) as smp, \
         tc.tile_pool(name="atp", bufs=4) as atp, \
         tc.tile_pool(name="yp", bufs=2) as yp, \
         tc.tile_pool(name="wdram", bufs=4, space="DRAM") as wdp, \
         tc.tile_pool(name="spsum", bufs=2, space="PSUM") as spp, \
         tc.tile_pool(name="ypsum", bufs=2, space="PSUM") as ypp, \
         tc.tile_pool(name="apsum", bufs=2, space="PSUM") as app:

        def emit_scores(qb):
            """scores -> exp -> renorm -> relu -> DRAM bounce -> aT tile.
            Both heads' weight strips land contiguously in one DRAM tile so
            a single xbar transpose yields [k, 2*nkt blocks, q]; AV then
            runs 256-wide with a strided rhs covering both heads."""
            klen = klens[qb]
            nkt = klen // 128
            nch = (klen + 1023) // 1024
            w_dram = wdp.tile([128, 2 * S], BF16, tag="w_dram")
            for h in range(HPC):
                p1 = sp.tile([128, S], BF16, tag=f"p1h{h}")
                p2 = sp.tile([128, S], BF16, tag=f"p2h{h}")
                r1c = smp.tile([128, 2], F32, tag=f"r1c{h}")
                r2c = smp.tile([128, 2], F32, tag=f"r2c{h}")
                q1T = q1r[h][:, qb * 128:(qb + 1) * 128]
                q2T = q2r[h][:, qb * 128:(qb + 1) * 128]
                for (qT, pstrip, rc) in ((q1T, p1, r1c), (q2T, p2, r2c)):
                    for c in range(nch):
                        k0 = c * 1024
                        kc = min(1024, klen - k0)
                        sps = spp.tile([128, 1024], F32, tag="s")
                        for sub in range(0, kc, 512):
                            sw = min(512, kc - sub)
                            nc.tensor.matmul(
                                sps[:, sub:sub + sw],
                                lhsT=qT,
                                rhs=kr[:, k0 + sub:k0 + sub + sw],
                                start=True,
                                stop=True,
                            )
                        if c == nch - 1:
                            # mask for the last (possibly partial) k block
                            dc = kc - 128
                            nc.vector.tensor_tensor(
                                out=sps[:, dc:dc + 128],
                                in0=sps[:, dc:dc + 128],
                                in1=mneg_s[:, qb * 128:(qb + 1) * 128],
                                op=OP.add,
                            )
                        nc.scalar.activation(
                            out=pstrip[:, k0:k0 + kc],
                            in_=sps[:, :kc],
                            func=AF.Exp,
                            accum_out=rc[:, c:c + 1],
                        )
                # rowsums. w = relu(p1/r1 - lam*p2/r2) is refactored as
                #   w = c1 * relu(p1 + c2r*p2),  c1 = 1/r1, c2r = -lam*r1/r2
                # so the whole renorm is ONE full-width stt pass plus a
                # fused relu+scale (ACT for h0, DVE two-scalar op for h1).
                c1 = smp.tile([128, 1], F32, tag=f"c1h{h}")
                c2r = smp.tile([128, 1], F32, tag=f"c2rh{h}")
                rec2 = smp.tile([128, 1], F32, tag=f"rec2h{h}")
                if nch > 1:
                    nc.vector.tensor_tensor(
                        out=r1c[:, 0:1], in0=r1c[:, 0:1], in1=r1c[:, 1:2], op=OP.add
                    )
                    nc.vector.tensor_tensor(
                        out=r2c[:, 0:1], in0=r2c[:, 0:1], in1=r2c[:, 1:2], op=OP.add
                    )
                nc.vector.reciprocal(out=c1, in_=r1c[:, 0:1])
                nc.vector.reciprocal(out=rec2, in_=r2c[:, 0:1])
                nc.vector.scalar_tensor_tensor(
                    out=c2r, in0=rec2, scalar=-LAM, in1=r1c[:, 0:1],
                    op0=OP.mult, op1=OP.mult,
                )
                nc.vector.scalar_tensor_tensor(
                    out=p2[:, :klen],
                    in0=p2[:, :klen],
                    scalar=c2r,
                    in1=p1[:, :klen],
                    op0=OP.mult,
                    op1=OP.add,
                )
                if h == 0:
                    nc.scalar.activation(out=p2[:, :klen], in_=p2[:, :klen],
                                         func=AF.Relu, scale=c1)
                else:
                    nc.vector.tensor_scalar(
                        out=p2[:, :klen], in0=p2[:, :klen],
                        scalar1=0.0, scalar2=c1, op0=OP.max, op1=OP.mult,
                    )
                # store this head's strip into its half of the shared DRAM
                # tile (gpsimd SWDGE queue — keeps scalar/sync free)
                nc.gpsimd.dma_start(out=w_dram[:, h * klen:h * klen + klen],
                                    in_=p2[:, :klen])
            # one batched xbar transpose for both heads: blocks 0..nkt-1 are
            # h0, nkt..2nkt-1 are h1
            aT = atp.tile([128, 2 * NQB, 128], BF16, tag="aT")
            nc.sync.dma_start(out=aT[:, :2 * nkt, :], in_=w_dram[:, :2 * klen],
                              transpose=True)
            return aT

        def emit_av_outproj(qb, aT):
            klen = klens[qb]
            nkt = klen // 128
            # attn_out^T[d, q] = sum_k v[k, d] * w^T[k, q], both heads in one
            # 256-wide moving operand: rhs = blocks (kt, nkt+kt) via stride
            avps = app.tile([128, 2, 128], F32, tag="av")
            aTb = aT[:, :2 * nkt, :].rearrange("p (h t) q -> p h t q", h=2)
            for kt in range(nkt):
                nc.tensor.matmul(
                    avps,
                    lhsT=v_nat[:, kt, :],
                    rhs=aTb[:, :, kt, :],
                    start=(kt == 0),
                    stop=(kt == nkt - 1),
                )
            at_s = smp.tile([128, 2, 128], BF16, name="attnT", tag="attnT")
            nc.vector.tensor_copy(out=at_s, in_=avps)
            # row-parallel out_proj partial: y[qb] = sum_h attnT_h.T @ ow_h
            ysb = yp.tile([128, HID], BF16, tag="ysb")
            for ec in range(4):
                yps = ypp.tile([128, 512], F32, tag="yps")
                e0 = ec * 512
                for h in range(HPC):
                    nc.tensor.matmul(
                        yps,
                        lhsT=at_s[:, h, :],
                        rhs=ow_s[:, h, e0:e0 + 512],
                        start=(h == 0),
                        stop=(h == HPC - 1),
                    )
                if ec % 2 == 0:
                    nc.scalar.copy(out=ysb[:, e0:e0 + 512], in_=yps)
                else:
                    nc.vector.tensor_copy(out=ysb[:, e0:e0 + 512], in_=yps)
            nc.scalar.dma_start(out=y[qb * 128:(qb + 1) * 128, :], in_=ysb)

        DEFER = 3
        pending = []
        for qb in range(NQB):
            aTs = emit_scores(qb)
            pending.append((qb, aTs))
            if len(pending) > DEFER:
                emit_av_outproj(*pending.pop(0))
        for item in pending:
            emit_av_outproj(*item)


def _build(klens):
    nc = bacc.Bacc("TRN2", target_bir_lowering=False, debug=False)
    xT = nc.dram_tensor("xT", [HID, S], BF16, kind="ExternalInput").ap()
    wq1 = nc.dram_tensor("wq1", [HID, HPC * D], BF16, kind="ExternalInput").ap()
    wq2 = nc.dram_tensor("wq2", [HID, HPC * D], BF16, kind="ExternalInput").ap()
    wk = nc.dram_tensor("wk", [HID, D], BF16, kind="ExternalInput").ap()
    wv = nc.dram_tensor("wv", [HID, D], BF16, kind="ExternalInput").ap()
    cosT = nc.dram_tensor("cosT", [D, S], BF16, kind="ExternalInput").ap()
    prot = nc.dram_tensor("prot", [D, D], BF16, kind="ExternalInput").ap()
    sinT = nc.dram_tensor("sinT", [D, S], BF16, kind="ExternalInput").ap()
    mneg = nc.dram_tensor("mneg", [128, S], F32, kind="ExternalInput").ap()
    ow = nc.dram_tensor("ow", [HPC * D, HID], BF16, kind="ExternalInput").ap()
    y = nc.dram_tensor("y", [S, HID], BF16, kind="ExternalOutput").ap()
    with ExitStack() as ctx:
        tc = ctx.enter_context(tile.TileContext(nc))
        _emit(ctx, tc, xT, wq1, wq2, wk, wv, cosT, sinT, prot, mneg, ow, y, klens)
    nc.compile()
    return nc


_RUNNER_CACHE = {}
LAST_RUN = None
LAST_EXEC = None  # (runner, dev_args) for timing reuse


class _Runner:
    """Mirrors bass2jax.run_bass_via_pjrt's multi-core path, but caches the
    jitted executable and keeps inputs reusable (no donation) so repeated
    timed executions don't recompile or re-upload."""

    def __init__(self, nc, n_cores):
        import jax
        from jax.sharding import Mesh, PartitionSpec
        from jax.experimental.shard_map import shard_map
        from concourse import bass2jax, mybir as mb

        bass2jax.install_neuronx_cc_hook()
        self.nc = nc
        self.n_cores = n_cores
        partition_name = (
            nc.partition_id_tensor.name if nc.partition_id_tensor else None
        )
        in_names, out_names, out_avals, zero_outs = [], [], [], []
        for alloc in nc.m.functions[0].allocations:
            if not isinstance(alloc, mb.MemoryLocationSet):
                continue
            name = alloc.memorylocations[0].name
            if alloc.kind == "ExternalInput":
                if name != partition_name:
                    in_names.append(name)
            elif alloc.kind == "ExternalOutput":
                out_names.append(name)
                shape = tuple(alloc.tensor_shape)
                dtype = mb.dt.np(alloc.dtype)
                out_avals.append(jax.core.ShapedArray(shape, dtype))
                zero_outs.append(np.zeros(shape, dtype))
        self.in_names = list(in_names)
        self.out_names = out_names
        self.out_avals = out_avals
        self.zero_outs = zero_outs
        n_params = len(in_names)
        all_names = list(in_names + out_names)
        if partition_name is not None:
            all_names.append(partition_name)
        all_names = tuple(all_names)

        def _body(*args):
            operands = list(args)
            if partition_name is not None:
                operands.append(bass2jax.partition_id_tensor())
            outs = bass2jax._bass_exec_p.bind(
                *operands,
                out_avals=tuple(out_avals),
                in_names=all_names,
                out_names=tuple(out_names),
                lowering_input_output_aliases=(),
                sim_require_finite=True,
                sim_require_nnan=True,
                nc=nc,
            )
            return tuple(outs)

        self._body = _body
        devices = jax.devices()[:n_cores]
        self.mesh = Mesh(np.asarray(devices), ("core",))
        self.pspec = PartitionSpec("core")
        in_specs = (self.pspec,) * (n_params + len(out_names))
        out_specs = (self.pspec,) * len(out_names)
        self.fn = jax.jit(
            shard_map(_body, mesh=self.mesh, in_specs=in_specs,
                      out_specs=out_specs, check_rep=False),
            keep_unused=True,
        )

    def device_args(self, in_maps):
        import jax
        from jax.sharding import NamedSharding

        sharding = NamedSharding(self.mesh, self.pspec)
        concat = [
            np.concatenate([np.asarray(m[name]) for m in in_maps], axis=0)
            for name in self.in_names
        ]
        concat += [
            np.zeros((self.n_cores * z.shape[0], *z.shape[1:]), z.dtype)
            for z in self.zero_outs
        ]
        return [jax.device_put(a, sharding) for a in concat]

    def run(self, dev_args):
        import jax

        outs = self.fn(*dev_args)
        jax.block_until_ready(outs)
        return [
            {
                name: np.asarray(outs[i]).reshape(
                    self.n_cores, *self.out_avals[i].shape)[c]
                for i, name in enumerate(self.out_names)
            }
            for c in range(self.n_cores)
        ]


def _get_runner(klens):
    key = tuple(klens)
    if key not in _RUNNER_CACHE:
        _RUNNER_CACHE[key] = _Runner(_build(klens), NCORES)
    return _RUNNER_CACHE[key]


def measure_hw(outdir="/tmp/kprof"):
    """Honest per-execution HW time: capture a neuron-profile NTFF of one
    kernel execution via the axon profiling ABI and report the profiler's
    exec time for the slowest profiled core."""
    import ctypes
    import glob
    import os
    import jax

    runner, dev_args = LAST_EXEC
    os.makedirs(outdir, exist_ok=True)
    for f in glob.glob(os.path.join(outdir, "*")):
        os.remove(f)

    lib = ctypes.CDLL('/opt/axon/libaxon_pjrt.so')
    lib.axon_start_nrt_profile.argtypes = [
        ctypes.POINTER(ctypes.c_int64), ctypes.c_size_t]
    lib.axon_start_nrt_profile.restype = ctypes.c_int64
    lib.axon_stop_nrt_profile.argtypes = [ctypes.c_char_p]
    lib.axon_stop_nrt_profile.restype = ctypes.c_int64

    # warm once (clocks/рstate), then profile a single execution
    jax.block_until_ready(runner.fn(*dev_args))
    rc = lib.axon_start_nrt_profile(None, 0)
    assert rc == 0, f"axon_start_nrt_profile rc={rc}"
    outs = runner.fn(*dev_args)
    jax.block_until_ready(outs)
    n = lib.axon_stop_nrt_profile(outdir.encode())
    assert n > 0, f"profile produced no files (rc={n})"

    from gauge.profiler import Profile
    from concourse._compat import FishPath

    prof = Profile(
        profile_path=FishPath(outdir),
        kernel_dev_mode=True,
        profile_on_exit=False,
        bass_kernel=runner.nc.m,
        offline_processing=True,
        fname="*",
    )
    results = prof.to_perfetto(model_index=(0,))
    exec_ns = max(r.exec_time_ns for r in results)
    trace_path = results[0].trace_path
    return exec_ns, trace_path


def _prep_mask(mask):
    """Per query-block: attended k extent (klen) and the additive mask for
    the last 128-wide k block. Requires every non-final block in range to
    be all-True (holds for causal and for all-ones masks)."""
    mask = np.asarray(mask).astype(bool)
    klens = []
    mneg = np.zeros((128, S), np.float32)
    for qb in range(NQB):
        rows = mask[qb * 128:(qb + 1) * 128, :]
        any_col = rows.any(axis=0)
        assert any_col.any(), f"query block {qb} attends nothing"
        last = int(np.nonzero(any_col)[0][-1])
        nkt = last // 128 + 1
        klen = nkt * 128
        klens.append(klen)
        blk = rows[:, (nkt - 1) * 128:klen]
        mneg[:, qb * 128:(qb + 1) * 128] = np.where(blk, 0.0, NEG_MASK)
        inner = rows[:, :(nkt - 1) * 128]
        if not inner.all():
            raise NotImplementedError(
                "mask has partial blocks before the final attended block; "
                "only causal / all-ones style masks are supported"
            )
    return klens, mneg


def host_prep(x, freqs_cos, freqs_sin, mask, q1_w, q2_w, k_w, v_w, out_w):
    """Host-side input marshalling: transpose/fold/shard. Returns
    (klens, in_maps)."""
    x = np.asarray(x, np.float32)
    assert x.shape == (1, S, HID)
    xT = np.ascontiguousarray(x[0].T)
    scale = 1.0 / math.sqrt(D)
    Z = (1.0 - LAM) + 1e-8

    cosT = np.ascontiguousarray(np.asarray(freqs_cos, np.float32).T)
    sinT = np.ascontiguousarray(np.asarray(freqs_sin, np.float32).T)
    # signed rotate-half as a matmul: rot = protM @ q with
    # protM[d, d+64] = -1 (d<64), protM[d, d-64] = +1 (d>=64); lhsT = protM.T
    protM = np.zeros((D, D), np.float32)
    for d in range(64):
        protM[d, d + 64] = -1.0
        protM[d + 64, d] = 1.0
    protT = np.ascontiguousarray(protM.T)

    klens, mneg = _prep_mask(mask)

    q1_w = np.asarray(q1_w, np.float32) * scale
    q2_w = np.asarray(q2_w, np.float32) * scale
    k_w = np.ascontiguousarray(np.asarray(k_w, np.float32))
    v_w = np.ascontiguousarray(np.asarray(v_w, np.float32) / Z)
    out_w = np.asarray(out_w, np.float32)

    import ml_dtypes
    bf = ml_dtypes.bfloat16
    xT = xT.astype(bf)
    k_w = k_w.astype(bf)
    v_w = v_w.astype(bf)
    protT = protT.astype(bf)
    cosT = cosT.astype(bf)
    sinT = sinT.astype(bf)
    in_maps = []
    for c in range(NCORES):
        h0 = c * HPC * D
        in_maps.append({
            "xT": xT,
            "wq1": np.ascontiguousarray(q1_w[:, h0:h0 + HPC * D]).astype(bf),
            "wq2": np.ascontiguousarray(q2_w[:, h0:h0 + HPC * D]).astype(bf),
            "wk": k_w,
            "wv": v_w,
            "cosT": cosT,
            "sinT": sinT,
            "prot": protT,
            "mneg": mneg,
            "ow": np.ascontiguousarray(out_w[h0:h0 + HPC * D, :]).astype(bf),
        })
    return klens, in_maps


def kernel(x, freqs_cos, freqs_sin, mask, q1_w, q2_w, k_w, v_w, out_w):
    global LAST_RUN, LAST_EXEC
    klens, in_maps = host_prep(
        x, freqs_cos, freqs_sin, mask, q1_w, q2_w, k_w, v_w, out_w)
    runner = _get_runner(klens)
    dev_args = runner.device_args(in_maps)
    LAST_EXEC = (runner, dev_args)
    results = runner.run(dev_args)
    LAST_RUN = results
    y = results[0]["y"].astype(np.float32)
    for c in range(1, NCORES):
        y = y + results[c]["y"].astype(np.float32)
    return y.reshape(1, S, HID)
